# revision 10
# baseline (speedup 1.0000x reference)
"""3-layer GraphSAGE (mean agg) on 8 trn2 NeuronCores.

Sharding: nodes data-parallel (6250/core, by dst owner), weights replicated.
Per core: edges with dst in its node range, sorted group-major by
(block-group, src-half, dst-block), padded to core-invariant per-(block,run)
window counts so one SPMD program works for all cores.

Pipeline per layer: one big dma_gather per (group, src-half) fetches edge
src rows (bf16) from HBM; per-group one-hot P matrices built with a single
batched is_equal (broadcast APs) on DVE; per 128-edge window a bf16 matmul
accumulates aggT[feat, slot] in PSUM; invdeg scale on DVE; dense layer
computed transposed (poT = Wn^T aggT + Ws^T hT, bias+relu fused on ACT) so
no transpose is needed to produce next layer's hT. A PE transpose produces
the node-major bf16 copy for the inter-layer AllGather / next gather.
"""

import os
import sys

sys.path.insert(0, "/opt/trn_rl_repo")

import numpy as np
import ml_dtypes

N_NODES = 50000
N_EDGES = 800000
DIM = 128
N_LAYERS = 3
CORES = 8
NPC = N_NODES // CORES          # 6250 nodes per core
BLK = 128
NBLK = (NPC + BLK - 1) // BLK   # 49 blocks (last has 106 valid rows)
NPC_PAD = NBLK * BLK            # 6272
HALF = N_NODES // 2             # 25000 (int16 gather index limit is 32767)
GB = 5                          # blocks per gather group

# AllGather split into K chunks (by dst-block ranges) so collectives overlap
# tail compute. hg / xbf rows are laid out chunk-major: for chunk k the
# AllGather output region holds [core0 rows, .., core7 rows] contiguously.
CHUNK_BSTART = [0, 13, 25, 37]
CHUNK_BEND = [13, 25, 37, 49]
CHUNK_ROWS = [
    min(e * BLK, NPC) - s * BLK for s, e in zip(CHUNK_BSTART, CHUNK_BEND)
]
CHUNK_CUM = np.concatenate([[0], np.cumsum(CHUNK_ROWS)]).astype(np.int64)


def _node_remap():
    """newid[u]: row of node u in the chunk-major hg/xbf layout."""
    u = np.arange(N_NODES, dtype=np.int64)
    c, j = u // NPC, u % NPC
    blk = j // BLK
    k = np.searchsorted(np.asarray(CHUNK_BEND), blk, side="right")
    rows_k = np.asarray(CHUNK_ROWS)[k]
    bstart = np.asarray(CHUNK_BSTART)[k]
    return CORES * CHUNK_CUM[k] + c * rows_k + (j - BLK * bstart)

LAST_EXEC_NS = [None]
LAST_PROFILE = [None]


def _schedule(W):
    """Group-major window schedule shared by host packing and program build.

    Returns (groups, woff, nwg) where groups is a list of block lists,
    woff[b, r] is the global window offset of (block b, run r), and
    nwg[g] is the number of windows in group g (both runs).
    """
    groups = [list(range(s, min(s + GB, NBLK))) for s in range(0, NBLK, GB)]
    woff = np.zeros((NBLK, 2), np.int64)
    nwg = []
    w = 0
    for blocks in groups:
        w0 = w
        for r in range(2):
            for b in blocks:
                woff[b, r] = w
                w += int(W[b, r])
        nwg.append(w - w0)
    assert w == int(W.sum())
    return groups, woff, nwg


def _preprocess(src, dst):
    """Host-side graph preprocessing. Returns the core-invariant window count
    table W plus per-core (idx128, dslotT) arrays in group-major order."""
    src = np.asarray(src).astype(np.int64)
    dst = np.asarray(dst).astype(np.int64)

    owner = dst // NPC
    run = (src >= HALF).astype(np.int64)
    dloc = dst - owner * NPC
    blk = dloc // BLK

    counts = np.zeros((CORES, NBLK, 2), np.int64)
    np.add.at(counts, (owner, blk, run), 1)
    # core-invariant window counts per (block, run)
    W = np.maximum(1, -(-counts.max(axis=0) // BLK))  # [NBLK, 2] ceil-div
    nw_total = int(W.sum())

    groups, woff, nwg = _schedule(W)

    per_core = []
    for c in range(CORES):
        sel = owner == c
        es, eb, er, ed = src[sel], blk[sel], run[sel], dloc[sel]
        order = np.lexsort((eb, er))
        es, eb, er, ed = es[order], eb[order], er[order], ed[order]
        # start offset of each (r, b) bucket in the sorted arrays
        bucket_off = np.zeros((NBLK, 2), np.int64)
        pos = 0
        for r in range(2):
            for b in range(NBLK):
                bucket_off[b, r] = pos
                pos += int(counts[c, b, r])
        assert pos == len(es)

        idx_out = np.zeros(nw_total * BLK, np.int16)
        dslot_out = np.full(nw_total * BLK, 255.0, np.float32)
        for r in range(2):
            for b in range(NBLK):
                cnt = int(counts[c, b, r])
                e0 = int(bucket_off[b, r])
                o0 = int(woff[b, r]) * BLK
                idx_out[o0:o0 + cnt] = (es[e0:e0 + cnt] - r * HALF).astype(np.int16)
                dslot_out[o0:o0 + cnt] = (ed[e0:e0 + cnt] - b * BLK).astype(np.float32)
                # pads: idx 0 (valid row, gathered but zeroed by P)

        # wrap indices in 16 partitions, replicate to 128 (one copy / Q7 core)
        wrapped = idx_out.reshape(-1, 16).T.copy()        # [16, nw_total*8]
        idx128 = np.tile(wrapped, (8, 1))                 # [128, nw_total*8]
        # dslotT: [128, nw_total]; column w = dslots of window w's 128 edges
        dslotT = dslot_out.reshape(nw_total, BLK).T.astype(ml_dtypes.bfloat16)
        per_core.append((idx128, dslotT))

    return W, per_core


def _build_program(W):
    import concourse.bass as bass
    import concourse.mybir as mybir
    import concourse.tile as tile
    from concourse import bacc

    f32 = mybir.dt.float32
    bf16 = mybir.dt.bfloat16
    i16 = mybir.dt.int16

    nw_total = int(W.sum())
    groups, woff, nwg = _schedule(W)

    nc = bacc.Bacc(
        "TRN2",
        target_bir_lowering=False,
        num_devices=CORES,
        num_swdge_queues=4,
    )

    # I/O
    xbf = nc.declare_dram_parameter("xbf", [N_NODES, DIM], bf16, isOutput=False)
    xT_in = nc.declare_dram_parameter("xT", [DIM, NPC_PAD], bf16, isOutput=False)
    idx_in = nc.declare_dram_parameter("idx", [128, nw_total * 8], i16, isOutput=False)
    dslot_in = nc.declare_dram_parameter("dslot", [128, nw_total], bf16, isOutput=False)
    invdeg_in = nc.declare_dram_parameter("invdeg", [128, NPC_PAD], f32, isOutput=False)
    ws_in = nc.declare_dram_parameter("Wself", [N_LAYERS * DIM, DIM], bf16, isOutput=False)
    wn_in = nc.declare_dram_parameter("Wneigh", [N_LAYERS * DIM, DIM], bf16, isOutput=False)
    b_in = nc.declare_dram_parameter("biascol", [DIM, N_LAYERS], f32, isOutput=False)
    iota_in = nc.declare_dram_parameter("iota", [128, 128], bf16, isOutput=False)
    ident_in = nc.declare_dram_parameter("ident", [128, 128], bf16, isOutput=False)
    # output: block-major [b, feat, slot]; host transposes to [node, feat]
    out_ext = nc.declare_dram_parameter("out", [NBLK * DIM, BLK], f32, isOutput=True)

    # internal DRAM for collectives
    hown = [nc.dram_tensor(f"hown{l}", [NPC, DIM], bf16) for l in range(2)]
    hg = [
        nc.dram_tensor(f"hg{l}", [N_NODES, DIM], bf16, addr_space="Shared")
        for l in range(2)
    ]
    rg = [list(range(CORES))]

    with tile.TileContext(nc) as tc:
        with (
            tc.tile_pool(name="persist", bufs=1) as pp,
            tc.tile_pool(name="msg", bufs=3) as msgp,
            tc.tile_pool(name="pwin", bufs=2) as pwp,
            tc.tile_pool(name="work", bufs=4) as wkp,
            tc.tile_pool(name="psA", bufs=2, space="PSUM") as psA,
            tc.tile_pool(name="psB", bufs=2, space="PSUM") as psB,
            tc.tile_pool(name="psT", bufs=2, space="PSUM") as psT,
        ):
            # --- persistent SBUF loads ---
            def load(shape, dt, src_ap, tag):
                t = pp.tile(shape, dt, tag=tag, name=tag)
                nc.sync.dma_start(out=t[:], in_=src_ap)
                return t

            idx_t = load([128, nw_total * 8], i16, idx_in[:, :], "idx")
            dslot_t = load([128, nw_total], bf16, dslot_in[:, :], "dslot")
            invdeg_t = load([128, NPC_PAD], f32, invdeg_in[:, :], "invdeg")
            iota_t = load([128, 128], bf16, iota_in[:, :], "iota")
            ident_t = load([128, 128], bf16, ident_in[:, :], "ident")
            ws_t = [
                load([128, DIM], bf16, ws_in[l * DIM : (l + 1) * DIM, :], f"ws{l}")
                for l in range(N_LAYERS)
            ]
            wn_t = [
                load([128, DIM], bf16, wn_in[l * DIM : (l + 1) * DIM, :], f"wn{l}")
                for l in range(N_LAYERS)
            ]
            biascol_t = load([DIM, N_LAYERS], f32, b_in[:, :], "biascol")

            # h transposed (bf16) for the self path; ping-pong buffers
            hT = [
                load([DIM, NPC_PAD], bf16, xT_in[:, :], "hT0"),
                pp.tile([DIM, NPC_PAD], bf16, tag="hT1", name="hT1"),
            ]

            gq = [0]  # SWDGE queue round-robin counter
            for l in range(N_LAYERS):
                src_dram = xbf if l == 0 else hg[l - 1]
                hT_cur = hT[l % 2]
                hT_next = hT[(l + 1) % 2]
                for g, blocks in enumerate(groups):
                    gw0 = int(woff[blocks[0], 0])
                    nw_g = int(nwg[g])
                    CH = int(os.environ.get("GNN_CH", "1024"))  # SWDGE ring cap
                    msg_t = msgp.tile([128, nw_g * DIM], bf16, tag="msg", name="msg")
                    for r in range(2):
                        w0 = int(woff[blocks[0], r])
                        nw_r = int(sum(W[b, r] for b in blocks))
                        lo = w0 - gw0          # local window offset in group
                        nidx = nw_r * BLK
                        step = nidx if CH == 0 else CH
                        for s0 in range(0, nidx, step):
                            n = min(step, nidx - s0)
                            nc.gpsimd.dma_gather(
                                out_ap=msg_t[
                                    :, lo * DIM + s0 : lo * DIM + s0 + n
                                ].rearrange("p (w e) -> p w e", e=DIM),
                                in_ap=src_dram[r * HALF : (r + 1) * HALF, :],
                                idxs_ap=idx_t[
                                    :, w0 * 8 + s0 // 16 : w0 * 8 + (s0 + n) // 16
                                ],
                                num_idxs=n,
                                num_idxs_reg=n,
                                elem_size=DIM,
                                elem_step=DIM,
                                queue_num=gq[0] % 4,
                            )
                            gq[0] += 1

                    # one-hot build: P[e, w, s] = (dslot[e, w] == iota[e, s])
                    P_t = pwp.tile([128, nw_g * BLK], bf16, tag="P", name="P")
                    if os.environ.get("GNN_NAIVE_P", "0") == "1":
                        for w in range(nw_g):
                            nc.vector.tensor_scalar(
                                out=P_t[:, w * BLK : (w + 1) * BLK],
                                in0=iota_t[:],
                                scalar1=dslot_t[:, gw0 + w : gw0 + w + 1],
                                scalar2=None,
                                op0=mybir.AluOpType.is_equal,
                            )
                    else:
                        nc.vector.tensor_tensor(
                            out=P_t[:].rearrange("p (w s) -> p w s", s=BLK),
                            in0=dslot_t[:, gw0 : gw0 + nw_g]
                            .unsqueeze(2)
                            .broadcast_to([128, nw_g, BLK]),
                            in1=iota_t[:, :]
                            .unsqueeze(1)
                            .broadcast_to([128, nw_g, BLK]),
                            op=mybir.AluOpType.is_equal,
                        )

                    for b in blocks:
                        pa = psA.tile([128, 128], f32, tag="agg", name="agg")
                        nwin_b = int(W[b, 0] + W[b, 1])
                        wi = 0
                        for r in range(2):
                            for k in range(int(W[b, r])):
                                wl = int(woff[b, r]) + k - gw0  # window in group
                                nc.tensor.matmul(
                                    pa[:],
                                    lhsT=msg_t[:, wl * DIM : (wl + 1) * DIM],
                                    rhs=P_t[:, wl * BLK : (wl + 1) * BLK],
                                    start=(wi == 0),
                                    stop=(wi == nwin_b - 1),
                                )
                                wi += 1
                        # aggT scaled by 1/deg (psum -> sbuf fused), bf16 out
                        aggT = wkp.tile([128, 128], bf16, tag="aggT", name="aggT")
                        nc.vector.tensor_tensor(
                            out=aggT[:],
                            in0=pa[:],
                            in1=invdeg_t[:, b * BLK : (b + 1) * BLK],
                            op=mybir.AluOpType.mult,
                        )
                        # dense, transposed: poT = Wn^T aggT + Ws^T hT_blk
                        po = psB.tile([128, 128], f32, tag="out", name="outp")
                        nc.tensor.matmul(
                            po[:], lhsT=wn_t[l][:], rhs=aggT[:],
                            start=True, stop=False,
                        )
                        nc.tensor.matmul(
                            po[:], lhsT=ws_t[l][:],
                            rhs=hT_cur[:, b * BLK : (b + 1) * BLK],
                            start=False, stop=True,
                        )
                        rows = min(BLK, NPC - b * BLK)
                        if l < N_LAYERS - 1:
                            # relu(poT + bias) -> hT_next (bf16), bias per-partition
                            nc.scalar.activation(
                                hT_next[:, b * BLK : (b + 1) * BLK],
                                po[:],
                                mybir.ActivationFunctionType.Relu,
                                bias=biascol_t[:, l : l + 1],
                            )
                            # node-major bf16 copy for AllGather / next gather
                            pt = psT.tile([128, 128], bf16, tag="tr", name="tr")
                            nc.tensor.transpose(
                                out=pt[:],
                                in_=hT_next[:, b * BLK : (b + 1) * BLK],
                                identity=ident_t[:],
                            )
                            hbf = wkp.tile([128, 128], bf16, tag="hbf", name="hbf")
                            nc.vector.tensor_copy(out=hbf[:], in_=pt[:])
                            nc.sync.dma_start(
                                out=hown[l][b * BLK : b * BLK + rows, :],
                                in_=hbf[:rows, :],
                            )
                        else:
                            oT = wkp.tile([128, 128], f32, tag="oT", name="oT")
                            nc.scalar.activation(
                                oT[:],
                                po[:],
                                mybir.ActivationFunctionType.Identity,
                                bias=biascol_t[:, l : l + 1],
                            )
                            nc.sync.dma_start(
                                out=out_ext[b * DIM : (b + 1) * DIM, :],
                                in_=oT[:, :],
                            )
                    # fire chunk collectives as soon as their blocks are done
                    if l < N_LAYERS - 1:
                        for k in range(len(CHUNK_ROWS)):
                            if CHUNK_BEND[k] - 1 in blocks:
                                r0 = CHUNK_BSTART[k] * BLK
                                rk = CHUNK_ROWS[k]
                                o0 = CORES * int(CHUNK_CUM[k])
                                nc.gpsimd.collective_compute(
                                    "AllGather",
                                    mybir.AluOpType.bypass,
                                    replica_groups=rg,
                                    ins=[hown[l][r0 : r0 + rk, :]],
                                    outs=[hg[l][o0 : o0 + CORES * rk, :]],
                                )

    nc.compile()
    return nc


def kernel(x, src, dst, W_self, W_neigh, b):
    from concourse.bass_utils import run_bass_kernel_spmd

    x = np.asarray(x, np.float32)
    W_self = np.asarray(W_self, np.float32)
    W_neigh = np.asarray(W_neigh, np.float32)
    b = np.asarray(b, np.float32)

    newid = _node_remap()
    src_m = newid[np.asarray(src).astype(np.int64)]
    W, per_core = _preprocess(src_m, dst)

    deg = np.bincount(np.asarray(dst).astype(np.int64), minlength=N_NODES)
    invdeg = (1.0 / np.maximum(deg, 1)).astype(np.float32)

    nc = _build_program(W)

    xg = np.empty_like(x)
    xg[newid] = x
    xbf = xg.astype(ml_dtypes.bfloat16)
    iota = np.tile(np.arange(128, dtype=np.float32), (128, 1)).astype(
        ml_dtypes.bfloat16
    )
    ident = np.eye(128, dtype=np.float32).astype(ml_dtypes.bfloat16)
    ws_flat = W_self.reshape(N_LAYERS * DIM, DIM).astype(ml_dtypes.bfloat16)
    wn_flat = W_neigh.reshape(N_LAYERS * DIM, DIM).astype(ml_dtypes.bfloat16)
    biascol = b.T.copy().astype(np.float32)  # [DIM, N_LAYERS]

    in_maps = []
    for c in range(CORES):
        idx128, dslotT = per_core[c]
        inv_c = np.zeros(NPC_PAD, np.float32)
        inv_c[:NPC] = invdeg[c * NPC : (c + 1) * NPC]
        inv_bc = np.tile(inv_c, (128, 1))
        xT = np.zeros((DIM, NPC_PAD), np.float32)
        xT[:, :NPC] = x[c * NPC : (c + 1) * NPC].T
        in_maps.append(
            {
                "xbf": xbf,
                "xT": xT.astype(ml_dtypes.bfloat16),
                "idx": idx128,
                "dslot": dslotT,
                "invdeg": inv_bc,
                "Wself": ws_flat,
                "Wneigh": wn_flat,
                "biascol": biascol,
                "iota": iota,
                "ident": ident,
            }
        )

    trace = os.environ.get("GNN_TRACE", "0") == "1"
    if trace:
        try:
            import types

            import antenv

            if "antenv.axon_hooks" not in sys.modules:
                mod = types.ModuleType("antenv.axon_hooks")
                mod._HOOK = None

                def _set(h, _m=mod):
                    _m._HOOK = h

                def _get(_m=mod):
                    return _m._HOOK

                mod.set_axon_ntff_profile_hook = _set
                mod.get_axon_ntff_profile_hook = _get
                sys.modules["antenv.axon_hooks"] = mod
                antenv.axon_hooks = mod
            from trn_agent_boot.trn_boot import _ntff_profile_via_ctypes

            sys.modules["antenv.axon_hooks"].set_axon_ntff_profile_hook(
                _ntff_profile_via_ctypes("/opt/axon/libaxon_pjrt.so")
            )
        except Exception as e:  # profiling is best-effort
            print(f"ntff hook setup failed: {e}")
            trace = False
    res = run_bass_kernel_spmd(
        nc, in_maps, core_ids=list(range(CORES)), trace=trace
    )
    LAST_EXEC_NS[0] = res.exec_time_ns
    LAST_PROFILE[0] = res.profile_json

    outs = []
    for c in range(CORES):
        o = res.results[c]["out"].reshape(NBLK, DIM, BLK)
        outs.append(np.transpose(o, (0, 2, 1)).reshape(NPC_PAD, DIM)[:NPC])
    return np.concatenate(outs, axis=0).astype(np.float32)
